# revision 21
# baseline (speedup 1.0000x reference)
"""Pointer-network attention scores on 8 Trainium2 NeuronCores (v4).

Reference computation (per batch b):
    enc = x_encoder @ w1.T            # (Nd, C)
    dec = x_decoder @ w2.T            # (Ne, C)
    prod[e,d] = sum_k v[k] * tanh(dec[e,k] + enc[d,k])
    out = softmax(prod + log(mask + 1e-16), axis=-1)

tanh(s) ~= sum_m c_m sin(w_m s) with the 5-frequency lattice
  {b0, b1, 2*b1, b0+b1, 2*(b0+b1)}   (fit on |s|<=5.8, max err 7.9e-3)
so only the TWO bases (b0, b1) need ScalarE Sin ACTs (8 passes, all
direct from PSUM: |b1*x| <= 3.2 is inside the spline's exact range and
the cos bias -pi/2 stays within the range the v3 kernel validated).
The other three frequencies' sin/cos factors come from angle identities
on VectorE/GpSimd in bf16 (2x/4x DVE modes):
    sin2t = 2 s c    (2 folded into the dec-side scale)
    cos2t = 1 - 2 s^2
    sin(a+b) = s_a c_b + c_a s_b,  cos(a+b) = c_a c_b - s_a s_b
This halves the ScalarE critical path (v3: 16 Sin ACTs + 2 range wraps).

Input DMA: two packed blobs on the sync HWDGE queue ([xd|w2] then
[w1|xe]) - per-DMA latency (~2.2us trigger+DGE+sem) dominates transfer
time, so fewer/larger DMAs win.  No DMAs are triggered from the scalar
engine: a scalar-engine DMACopy before the first ACT makes the
table-load pass emit an extra default LoadActFuncSet (1.28us) ahead of
the trig table.  Small/late operands (vcn consts, mask, identity) ride
the gpsimd SWDGE queue.  A dozen dummy matmuls warm the PE p-state
(0.65 -> 2.4 GHz) during the DMA window.  The mask-bias matmul is the
LAST accumulation step per half so the mask DMA is never on the
critical path.

Sharding: data-parallel over (batch, decoder-half): core = 2*b + half.
Softmax axis (Nd) stays intact per core; no collectives.
"""

import math
from contextlib import ExitStack

import numpy as np

import concourse.bass as bass
import concourse.bacc as bacc
import concourse.mybir as mybir
import concourse.tile as tile
from concourse.bass_utils import run_bass_kernel_spmd

B, NE, ND, C = 4, 512, 512, 256
NCORES = 8
EH = NE // 2          # decoder rows per core
P = 128               # partitions

# tanh(s) ~= sum c_m sin(w_m s); w = [b0, b1, 2b1, b0+b1, 2(b0+b1)]
B0 = 0.39347
B1 = 1.10765
FREQS = [B0, B1, 2 * B1, B0 + B1, 2 * (B0 + B1)]
COEFS = [1.1887, 0.19858, 0.05098, 0.08935, 0.01576]

F32 = mybir.dt.float32
BF16 = mybir.dt.bfloat16

HALF_PI = float(np.float32(math.pi / 2))
# log(float32(1e-16)); the -36.84 shift common to all logits is dropped
MASK_SCALE = float(-np.log(np.float32(1e-16)))

Sin = mybir.ActivationFunctionType.Sin
Exp = mybir.ActivationFunctionType.Exp
MULT = mybir.AluOpType.mult
ADD = mybir.AluOpType.add
SUB = mybir.AluOpType.subtract


def _build_program(finalize=True):
    nc = bacc.Bacc(trn_type="TRN2", debug=False)

    # blob1 = [xd | w2] (pd inputs), blob2 = [w1 | xe] (pe inputs)
    blob1 = nc.declare_dram_parameter("blob1", [P, 1024], BF16, isOutput=False)
    blob2 = nc.declare_dram_parameter("blob2", [P, 1536], BF16, isOutput=False)
    vcn = nc.declare_dram_parameter("vcn", [P, 2, 5], F32, isOutput=False)
    msk = nc.declare_dram_parameter("msk", [P, 2, ND], BF16, isOutput=False)
    ident = nc.declare_dram_parameter("ident", [P, P], BF16, isOutput=False)
    out = nc.declare_dram_parameter("out", [P, 2, ND], BF16, isOutput=True)

    with tile.TileContext(nc) as tc, ExitStack() as ctx:
        const = ctx.enter_context(tc.tile_pool(name="const", bufs=1))
        persist = ctx.enter_context(tc.tile_pool(name="persist", bufs=1))
        wrk = ctx.enter_context(tc.tile_pool(name="wrk", bufs=2))
        psum = ctx.enter_context(tc.tile_pool(name="psum", bufs=1, space="PSUM"))

        # ---- input DMA ----
        b1_sb = const.tile([P, 1024], BF16, tag="b1_sb")
        b2_sb = const.tile([P, 1536], BF16, tag="b2_sb")
        vcn_sb = const.tile([P, 2, 5], F32, tag="vcn_sb")
        mk_sb = const.tile([P, 2, ND], BF16, tag="mk_sb")
        id_sb = const.tile([P, P], BF16, tag="id_sb")
        nc.sync.dma_start(out=b1_sb, in_=blob1.ap())
        nc.sync.dma_start(out=b2_sb, in_=blob2.ap())
        nc.gpsimd.dma_start(out=vcn_sb, in_=vcn.ap())
        nc.gpsimd.dma_start(out=mk_sb, in_=msk.ap())
        nc.gpsimd.dma_start(out=id_sb, in_=ident.ap())

        nhpi = const.tile([P, 1], F32, tag="nhpi")
        nc.vector.memset(nhpi, -HALF_PI)
        # first ScalarE ACT is a Sin so the trig table loads immediately
        warm = const.tile([P, 1], F32, tag="warm")
        nc.scalar.activation(warm, nhpi, Sin)

        # ---- PE p-state warmup (dummy matmuls on a memset tile) ----
        wz = const.tile([P, P], BF16, tag="wz")
        nc.vector.memset(wz, 0.0)
        pwarm = psum.tile([P, P], F32, tag="pwarm")
        for _ in range(22):
            nc.tensor.matmul(pwarm, lhsT=wz, rhs=wz,
                             start=True, stop=True)

        # ---- projections (bf16 matmul, f32 accum) ----
        pd = psum.tile([P, 2, EH], F32, tag="pd")    # [c_lo, kt, e]
        pe = psum.tile([P, 2, ND], F32, tag="pe")    # [c_lo, kt, d]
        for kt in range(2):
            for ct in range(2):
                nc.tensor.matmul(
                    pd[:, kt, :],
                    lhsT=b1_sb[:, 512 + ct * 256 + kt * P:
                               512 + ct * 256 + (kt + 1) * P],
                    rhs=b1_sb[:, ct * 256:(ct + 1) * 256],
                    start=(ct == 0), stop=(ct == 1),
                )
        for kt in range(2):
            for ct in range(2):
                nc.tensor.matmul(
                    pe[:, kt, :],
                    lhsT=b2_sb[:, ct * 256 + kt * P:ct * 256 + (kt + 1) * P],
                    rhs=b2_sb[:, 512 + ct * 512:512 + (ct + 1) * 512],
                    start=(ct == 0), stop=(ct == 1),
                )

        # ---- base factors (ScalarE, direct from PSUM) ----
        # layouts: decb [P, kt, slot, EH], encb [P, kt, slot, ND]
        # slots: 0=s0, 1=c0, 2=s1, 3=c1
        decb = persist.tile([P, 2, 4, EH], BF16, tag="decb")
        encb = persist.tile([P, 2, 4, ND], BF16, tag="encb")
        w0 = float(np.float32(FREQS[0]))
        w1_ = float(np.float32(FREQS[1]))
        # order: m0 dec, m1 dec (dec chains run while enc ACTs proceed),
        # then m1 enc, m0 enc.  The zero-bias tile zb (computed from the
        # last dec ACT's output) forces the scheduler to keep all dec ACTs
        # ahead of the enc ACTs - without it the static order interleaves
        # and ScalarE stalls on pe while dec work was ready.
        nc.scalar.activation(decb[:, :, 0, :], pd, Sin, scale=w0)
        nc.scalar.activation(decb[:, :, 1, :], pd, Sin, scale=w0, bias=nhpi)
        nc.scalar.activation(decb[:, :, 2, :], pd, Sin, scale=w1_)
        nc.scalar.activation(decb[:, :, 3, :], pd, Sin, scale=w1_, bias=nhpi)
        zb = const.tile([P, 1], F32, tag="zb")
        nc.scalar.activation(zb, decb[:, 0, 3, 0:1],
                             mybir.ActivationFunctionType.Copy, scale=0.0)
        nzhpi = const.tile([P, 1], F32, tag="nzhpi")
        nc.vector.tensor_scalar(nzhpi, zb, 1.0, -HALF_PI, op0=MULT, op1=ADD)
        nc.scalar.activation(encb[:, :, 0, :], pe, Sin, scale=w0, bias=zb)
        nc.scalar.activation(encb[:, :, 1, :], pe, Sin, scale=w0, bias=nzhpi)
        nc.scalar.activation(encb[:, :, 2, :], pe, Sin, scale=w1_, bias=zb)
        nc.scalar.activation(encb[:, :, 3, :], pe, Sin, scale=w1_, bias=nzhpi)
        # enc squares on ScalarE (idle after the base ACTs; Square is in
        # the trig table set so no table switch)
        Square = mybir.ActivationFunctionType.Square
        u1e = persist.tile([P, 2, ND], BF16, tag="u1e")
        nc.scalar.activation(u1e, encb[:, :, 2, :], Square)

        # NOTE: the ACT "cos" slots hold Sin(wx - pi/2) = -cos(wx); all the
        # sign flips are folded into the host-side vcn table columns:
        #   0: -c0*v   1: -c1*v   2: -2*c2*v   3: -c3*v   4: -2*c4*v
        def vc(kt, col):
            return vcn_sb[:, kt, col:col + 1]

        # ---- dec-side scaled factors ----
        # paS[m][sc]: [P, kt, EH] bf16, sc 0 pairs with enc-cos, 1 with enc-sin
        paS = persist.tile([P, 5, 2, 2, EH], BF16, tag="paS")

        for kt in range(2):
            nc.vector.tensor_scalar(paS[:, 0, kt, :, :], decb[:, kt, 0:2, :],
                                    vc(kt, 0), None, op0=MULT)
            nc.vector.tensor_scalar(paS[:, 1, kt, :, :], decb[:, kt, 2:4, :],
                                    vc(kt, 1), None, op0=MULT)

        # dec m2 = 2*b1: [u1|t12] = [s1|s1] * [s1|c1hat]
        #   u1 = sin^2, t12 = s*(-cos) = -sin(2t)/2
        dut = wrk.tile([P, 2, 2, EH], BF16, tag="dut")
        nc.vector.tensor_tensor(
            dut, decb[:, :, 2:3, :].broadcast_to([P, 2, 2, EH]),
            decb[:, :, 2:4, :], MULT)
        # dcos2 = cos(2t) = 1 - 2*u1 (true cosine; immediate TS is fast)
        dcos2 = wrk.tile([P, 2, EH], BF16, tag="dcos2")
        nc.vector.tensor_scalar(dcos2, dut[:, :, 0, :], -2.0, 1.0,
                                op0=MULT, op1=ADD)
        for kt in range(2):
            # sin2-scaled = t12*(-2v2) = v2*sin2 ; cos-side partner pairs
            # with enc t12e = -sin2b/2, so scale = -2v2 * cos2
            nc.vector.tensor_scalar(paS[:, 2, kt, 0, :], dut[:, kt, 1, :],
                                    vc(kt, 2), None, op0=MULT)
            nc.vector.tensor_scalar(paS[:, 2, kt, 1, :], dcos2[:, kt, :],
                                    vc(kt, 2), None, op0=MULT)

        # dec m3 = b0+b1: with chat = -cos slots,
        #   t1+t2 = s0*chat1 + chat0*s1 = -sin(S),  t3-t4 = chat0*chat1
        #   - s0*s1 = +cos(S)
        dt12 = wrk.tile([P, 2, 2, EH], BF16, tag="dt12")   # [t1|t2]
        dt43 = wrk.tile([P, 2, 2, EH], BF16, tag="dt43")   # [t4|t3]
        nc.vector.tensor_tensor(dt12[:, :, 0, :], decb[:, :, 0, :],
                                decb[:, :, 3, :], MULT)    # t1 = s0*chat1
        nc.vector.tensor_tensor(dt12[:, :, 1, :], decb[:, :, 1, :],
                                decb[:, :, 2, :], MULT)    # t2 = chat0*s1
        # [t4|t3] = [s0|chat0] * [s1|chat1]
        nc.vector.tensor_tensor(dt43, decb[:, :, 0:2, :],
                                decb[:, :, 2:4, :], MULT)
        dsc3 = wrk.tile([P, 2, 2, EH], BF16, tag="dsc3")   # [-sinS|+cosS]
        nc.vector.tensor_tensor(dsc3[:, :, 0, :], dt12[:, :, 0, :],
                                dt12[:, :, 1, :], ADD)
        nc.vector.tensor_tensor(dsc3[:, :, 1, :], dt43[:, :, 1, :],
                                dt43[:, :, 0, :], SUB)
        # dec m4 = 2*m3: [u3|t34] = [sin^2 S | -sin(2S)/2]
        dut3 = wrk.tile([P, 2, 2, EH], BF16, tag="dut3")
        nc.vector.tensor_tensor(
            dut3, dsc3[:, :, 0:1, :].broadcast_to([P, 2, 2, EH]),
            dsc3, MULT)
        dcos4 = wrk.tile([P, 2, EH], BF16, tag="dcos4")
        nc.vector.tensor_scalar(dcos4, dut3[:, :, 0, :], -2.0, 1.0,
                                op0=MULT, op1=ADD)
        for kt in range(2):
            nc.vector.tensor_scalar(paS[:, 3, kt, :, :], dsc3[:, kt, :, :],
                                    vc(kt, 3), None, op0=MULT)
            nc.vector.tensor_scalar(paS[:, 4, kt, 0, :], dut3[:, kt, 1, :],
                                    vc(kt, 4), None, op0=MULT)
            nc.vector.tensor_scalar(paS[:, 4, kt, 1, :], dcos4[:, kt, :],
                                    vc(kt, 4), None, op0=MULT)

        # ---- enc-side factors (true values; constant 2s folded into dec) ----
        # enc m2: t12e = s1*c1hat = -sin2/2 ; q2c = 1 - 2*u1e (u1e on ScalarE)
        t12e = persist.tile([P, 2, ND], BF16, tag="t12e")
        nc.vector.tensor_tensor(t12e, encb[:, :, 2, :],
                                encb[:, :, 3, :], MULT)
        q2c = persist.tile([P, 2, ND], BF16, tag="q2c")
        nc.vector.tensor_scalar(q2c, u1e, -2.0, 1.0, op0=MULT, op1=ADD)
        # enc m3: sum/diff chain
        et12 = wrk.tile([P, 2, 2, ND], BF16, tag="et12")
        et43 = wrk.tile([P, 2, 2, ND], BF16, tag="et43")
        nc.vector.tensor_tensor(et12[:, :, 0, :], encb[:, :, 0, :],
                                encb[:, :, 3, :], MULT)    # t1e = s0*c1
        nc.vector.tensor_tensor(et12[:, :, 1, :], encb[:, :, 1, :],
                                encb[:, :, 2, :], MULT)    # t2e = c0*s1
        nc.vector.tensor_tensor(et43, encb[:, :, 0:2, :],
                                encb[:, :, 2:4, :], MULT)  # [t4e|t3e]
        esc3 = persist.tile([P, 2, 2, ND], BF16, tag="esc3")  # [q3s|q3c]
        nc.vector.tensor_tensor(esc3[:, :, 0, :], et12[:, :, 0, :],
                                et12[:, :, 1, :], ADD)
        nc.vector.tensor_tensor(esc3[:, :, 1, :], et43[:, :, 1, :],
                                et43[:, :, 0, :], SUB)
        # enc m4: t34e = q3s*q3c = -sin(2S)/2 ; u3e = q3s^2 on ScalarE
        t34e = persist.tile([P, 2, ND], BF16, tag="t34e")
        nc.vector.tensor_tensor(t34e, esc3[:, :, 0, :],
                                esc3[:, :, 1, :], MULT)
        u3e = persist.tile([P, 2, ND], BF16, tag="u3e")
        nc.scalar.activation(u3e, esc3[:, :, 0, :], Square)
        q4c = persist.tile([P, 2, ND], BF16, tag="q4c")
        nc.vector.tensor_scalar(q4c, u3e, -2.0, 1.0, op0=MULT, op1=ADD)

        # ---- pair-product matmuls ----
        # per m: lhsT = paS[m,kt,0] x rhs = enc-cos ; paS[m,kt,1] x enc-sin
        # rhs (enc) tensors per m: (cos-factor, sin-factor)
        enc_rhs = [
            (encb[:, :, 1, :], encb[:, :, 0, :]),   # m0: c0hat, s0
            (encb[:, :, 3, :], encb[:, :, 2, :]),   # m1: c1hat, s1
            (q2c, t12e),                            # m2: cos2, -sin2/2
            (esc3[:, :, 1, :], esc3[:, :, 0, :]),   # m3: +cosS, -sinS
            (q4c, t34e),                            # m4: cos2S, -sin2S/2
        ]
        pbig = [psum.tile([P, ND], F32, tag=f"pbig{et}", name=f"pbig{et}")
                for et in range(2)]
        # et-major: half 0's chain completes before half 1's final pairs so
        # its softmax+store overlap them.  The mask bias OPENS each chain
        # (the mask lands ~6us before the first pair needs it) so the
        # chain's stop comes right after the last m4 pair.
        for et in range(2):
            nc.tensor.matmul(pbig[et], lhsT=id_sb, rhs=mk_sb[:, et, :],
                             start=True, stop=False)
        for et in range(2):
            for mi in range(5):
                rc, rs = enc_rhs[mi]
                for kt in range(2):
                    nc.tensor.matmul(
                        pbig[et],
                        lhsT=paS[:, mi, kt, 0, et * P:(et + 1) * P],
                        rhs=rc[:, kt, :],
                        start=False, stop=False,
                    )
                    nc.tensor.matmul(
                        pbig[et],
                        lhsT=paS[:, mi, kt, 1, et * P:(et + 1) * P],
                        rhs=rs[:, kt, :],
                        start=False,
                        stop=(mi == 4 and kt == 1),
                    )

        # preload the exp table set behind the final pair matmuls
        warm2 = const.tile([P, 1], F32, tag="warm2")
        nc.scalar.activation(warm2, encb[:, 0, 3, 0:1], Exp)

        # ---- masked softmax over d (free axis) ----
        for et in range(2):
            expv = wrk.tile([P, ND], F32, tag="expv", name=f"expv{et}")
            zsum = wrk.tile([P, 1], F32, tag="zsum", name=f"zsum{et}")
            nc.scalar.activation(expv, pbig[et], Exp, accum_out=zsum)
            rz = wrk.tile([P, 1], F32, tag="rz", name=f"rz{et}")
            nc.vector.reciprocal(rz, zsum)
            outv = wrk.tile([P, ND], BF16, tag="outv", name=f"outv{et}")
            nc.vector.tensor_scalar(outv, expv, rz, None, op0=MULT)
            nc.sync.dma_start(out=out.ap()[:, et, :], in_=outv)

    if finalize:
        nc.finalize()
    return nc


_PROGRAM = None


def _get_program():
    global _PROGRAM
    if _PROGRAM is None:
        _PROGRAM = _build_program()
    return _PROGRAM


def build_in_maps(x_decoder, x_encoder, mask, w1, w2, v):
    import ml_dtypes
    bf = ml_dtypes.bfloat16
    x_decoder = np.asarray(x_decoder, dtype=np.float32)
    x_encoder = np.asarray(x_encoder, dtype=np.float32)
    mask = np.asarray(mask)
    w1 = np.asarray(w1, dtype=np.float32)
    w2 = np.asarray(w2, dtype=np.float32)
    v = np.asarray(v, dtype=np.float32)

    def pm(mat, cols):
        """[C, cols] -> partition-major [P, 2, cols] (c = ct*128 + p)."""
        return np.ascontiguousarray(
            mat.reshape(2, P, cols).transpose(1, 0, 2)).astype(bf)

    w1p = pm(np.ascontiguousarray(w1.T), C)    # [P, ct, k]
    w2p = pm(np.ascontiguousarray(w2.T), C)
    xep_all = {}
    for b in range(B):
        xep_all[b] = pm(np.ascontiguousarray(x_encoder[b].T), ND)

    # vcn[p, kt, col], k = kt*128 + p; signs absorb the -cos ACT slots
    cf = [np.float32(c) for c in COEFS]
    vcn = np.empty((P, 2, 5), dtype=np.float32)
    for kt in range(2):
        vk = v[kt * P:(kt + 1) * P]
        vcn[:, kt, 0] = -cf[0] * vk
        vcn[:, kt, 1] = -cf[1] * vk
        vcn[:, kt, 2] = -2 * cf[2] * vk
        vcn[:, kt, 3] = -cf[3] * vk
        vcn[:, kt, 4] = -2 * cf[4] * vk

    identity = np.eye(P, dtype=np.float32).astype(bf)
    in_maps = []
    for core in range(NCORES):
        b, h = divmod(core, 2)
        sl = slice(h * EH, (h + 1) * EH)
        xdp = pm(np.ascontiguousarray(x_decoder[b, sl, :].T), EH)  # [P,ct,e]
        blob1 = np.concatenate(
            [xdp.reshape(P, 512), w2p.reshape(P, 512)], axis=1)
        blob2 = np.concatenate(
            [w1p.reshape(P, 512), xep_all[b].reshape(P, 1024)], axis=1)
        mskp = (mask[b, sl, :].astype(np.float32) * np.float32(MASK_SCALE)
                ).reshape(2, P, ND).transpose(1, 0, 2)  # e = et*128 + p
        in_maps.append({
            "blob1": np.ascontiguousarray(blob1),
            "blob2": np.ascontiguousarray(blob2),
            "vcn": vcn,
            "msk": np.ascontiguousarray(mskp).astype(bf),
            "ident": identity,
        })
    return in_maps


def kernel(x_decoder, x_encoder, mask, w1, w2, v):
    in_maps = build_in_maps(x_decoder, x_encoder, mask, w1, w2, v)
    nc = _get_program()
    res = run_bass_kernel_spmd(nc, in_maps, core_ids=list(range(NCORES)))

    out = np.empty((B, NE, ND), dtype=np.float32)
    for core in range(NCORES):
        b, h = divmod(core, 2)
        o = res.results[core]["out"].astype(np.float32)  # [P, 2, ND]
        out[b, h * EH:(h + 1) * EH, :] = \
            o.transpose(1, 0, 2).reshape(EH, ND)
    return out


# revision 23
# speedup vs baseline: 1.0125x; 1.0125x over previous
"""Pointer-network attention scores on 8 Trainium2 NeuronCores (v4).

Reference computation (per batch b):
    enc = x_encoder @ w1.T            # (Nd, C)
    dec = x_decoder @ w2.T            # (Ne, C)
    prod[e,d] = sum_k v[k] * tanh(dec[e,k] + enc[d,k])
    out = softmax(prod + log(mask + 1e-16), axis=-1)

tanh(s) ~= sum_m c_m sin(w_m s) with the 5-frequency lattice
  {b0, b1, 2*b1, b0+b1, 2*(b0+b1)}   (fit on |s|<=5.8, max err 7.9e-3)
so only the TWO bases (b0, b1) need ScalarE Sin ACTs (8 passes, all
direct from PSUM: |b1*x| <= 3.2 is inside the spline's exact range and
the cos bias -pi/2 stays within the range the v3 kernel validated).
The other three frequencies' sin/cos factors come from angle identities
in bf16 (VectorE 2x/4x DVE modes; two squares ride idle ScalarE):
    sin2t = 2 s c    (2 folded into the dec-side scale)
    cos2t = 1 - 2 s^2
    sin(a+b) = s_a c_b + c_a s_b,  cos(a+b) = c_a c_b - s_a s_b
This halves the ScalarE critical path (v3: 16 Sin ACTs + 2 range wraps)
at the cost of ~13us of VectorE bf16 tensor ops, which become the tail.
The ACT cos slots hold Sin(wx-pi/2) = -cos(wx); all sign flips are
folded into the host-side vcn table.  Hard-won scheduling notes:
 - gpsimd (Pool/Q7) tensor ops run ~15ns/col - never put vector work
   there; two-AP-scalar tensor_scalar also hits a ~15ns/col slow path
   (immediate-pair TS + single-AP TS are 4x/2x fast paths).
 - a scalar-engine DMACopy before the first ACT makes the table-load
   pass emit an extra default LoadActFuncSet (1.28us) ahead of the
   trig table, so no DMAs are ever triggered from nc.scalar.
 - the Tile scheduler's static per-engine order follows its cost model,
   not emission order: the zb zero-bias tile (computed from the last
   dec ACT) forces all dec ACTs ahead of enc ACTs, and the Exp-preload
   (warm2) reads the LAST enc ACT's output so the exp table load can
   never be scheduled between Sin ACTs (table thrash).
 - PE DVFS: ~22 dummy matmuls during the DMA window lift the clock to
   the mid p-state; sustained pair-matmul streams reach full rate.

Input DMA: two packed blobs on the sync HWDGE queue ([xd|w2] then
[w1|xe]) - per-DMA latency (~2.2us trigger+DGE+sem-prop) dominates
transfer time (16 shared DMA engines, 360GB/s), so fewer/larger DMAs
win.  Small/late operands (vcn consts, mask, identity) ride the gpsimd
SWDGE queue.  The mask-bias matmul is the LAST accumulation step per
half so the mask DMA is never on the critical path; outputs go back on
the sync HWDGE queue.

Sharding: data-parallel over (batch, decoder-half): core = 2*b + half.
Softmax axis (Nd) stays intact per core; no collectives.
"""

import math
from contextlib import ExitStack

import numpy as np

import concourse.bass as bass
import concourse.bacc as bacc
import concourse.mybir as mybir
import concourse.tile as tile
from concourse.bass_utils import run_bass_kernel_spmd

B, NE, ND, C = 4, 512, 512, 256
NCORES = 8
EH = NE // 2          # decoder rows per core
P = 128               # partitions

# tanh(s) ~= sum c_m sin(w_m s); w = [b0, b1, 2b1, b0+b1, 2(b0+b1)]
B0 = 0.39347
B1 = 1.10765
FREQS = [B0, B1, 2 * B1, B0 + B1, 2 * (B0 + B1)]
COEFS = [1.1887, 0.19858, 0.05098, 0.08935, 0.01576]

F32 = mybir.dt.float32
BF16 = mybir.dt.bfloat16

HALF_PI = float(np.float32(math.pi / 2))
# log(float32(1e-16)); the -36.84 shift common to all logits is dropped
MASK_SCALE = float(-np.log(np.float32(1e-16)))

Sin = mybir.ActivationFunctionType.Sin
Exp = mybir.ActivationFunctionType.Exp
MULT = mybir.AluOpType.mult
ADD = mybir.AluOpType.add
SUB = mybir.AluOpType.subtract


def _build_program(finalize=True):
    nc = bacc.Bacc(trn_type="TRN2", debug=False)

    # blob1 = [xd | w2] (pd inputs), blob2 = [w1 | xe] (pe inputs)
    blob1 = nc.declare_dram_parameter("blob1", [P, 1024], BF16, isOutput=False)
    blob2 = nc.declare_dram_parameter("blob2", [P, 1536], BF16, isOutput=False)
    vcn = nc.declare_dram_parameter("vcn", [P, 2, 5], F32, isOutput=False)
    msk = nc.declare_dram_parameter("msk", [P, 2, ND], BF16, isOutput=False)
    ident = nc.declare_dram_parameter("ident", [P, P], BF16, isOutput=False)
    out = nc.declare_dram_parameter("out", [P, 2, ND], BF16, isOutput=True)

    with tile.TileContext(nc) as tc, ExitStack() as ctx:
        const = ctx.enter_context(tc.tile_pool(name="const", bufs=1))
        persist = ctx.enter_context(tc.tile_pool(name="persist", bufs=1))
        wrk = ctx.enter_context(tc.tile_pool(name="wrk", bufs=2))
        psum = ctx.enter_context(tc.tile_pool(name="psum", bufs=1, space="PSUM"))

        # ---- input DMA ----
        b1_sb = const.tile([P, 1024], BF16, tag="b1_sb")
        b2_sb = const.tile([P, 1536], BF16, tag="b2_sb")
        vcn_sb = const.tile([P, 2, 5], F32, tag="vcn_sb")
        mk_sb = const.tile([P, 2, ND], BF16, tag="mk_sb")
        id_sb = const.tile([P, P], BF16, tag="id_sb")
        nc.sync.dma_start(out=b1_sb, in_=blob1.ap())
        nc.sync.dma_start(out=b2_sb, in_=blob2.ap())
        nc.gpsimd.dma_start(out=vcn_sb, in_=vcn.ap())
        nc.gpsimd.dma_start(out=mk_sb, in_=msk.ap())
        nc.gpsimd.dma_start(out=id_sb, in_=ident.ap())

        nhpi = const.tile([P, 1], F32, tag="nhpi")
        nc.vector.memset(nhpi, -HALF_PI)
        # first ScalarE ACT is a Sin so the trig table loads immediately
        warm = const.tile([P, 1], F32, tag="warm")
        nc.scalar.activation(warm, nhpi, Sin)

        # ---- PE p-state warmup (dummy matmuls on a memset tile) ----
        wz = const.tile([P, P], BF16, tag="wz")
        nc.vector.memset(wz, 0.0)
        pwarm = psum.tile([P, P], F32, tag="pwarm")
        for _ in range(22):
            nc.tensor.matmul(pwarm, lhsT=wz, rhs=wz,
                             start=True, stop=True)

        # ---- projections (bf16 matmul, f32 accum) ----
        pd = psum.tile([P, 2, EH], F32, tag="pd")    # [c_lo, kt, e]
        pe = psum.tile([P, 2, ND], F32, tag="pe")    # [c_lo, kt, d]
        for kt in range(2):
            for ct in range(2):
                nc.tensor.matmul(
                    pd[:, kt, :],
                    lhsT=b1_sb[:, 512 + ct * 256 + kt * P:
                               512 + ct * 256 + (kt + 1) * P],
                    rhs=b1_sb[:, ct * 256:(ct + 1) * 256],
                    start=(ct == 0), stop=(ct == 1),
                )
        for kt in range(2):
            for ct in range(2):
                nc.tensor.matmul(
                    pe[:, kt, :],
                    lhsT=b2_sb[:, ct * 256 + kt * P:ct * 256 + (kt + 1) * P],
                    rhs=b2_sb[:, 512 + ct * 512:512 + (ct + 1) * 512],
                    start=(ct == 0), stop=(ct == 1),
                )

        # ---- base factors (ScalarE, direct from PSUM) ----
        # layouts: decb [P, kt, slot, EH], encb [P, kt, slot, ND]
        # slots: 0=s0, 1=c0, 2=s1, 3=c1
        decb = persist.tile([P, 2, 4, EH], BF16, tag="decb")
        encb = persist.tile([P, 2, 4, ND], BF16, tag="encb")
        w0 = float(np.float32(FREQS[0]))
        w1_ = float(np.float32(FREQS[1]))
        # order: m0 dec, m1 dec (dec chains run while enc ACTs proceed),
        # then m1 enc, m0 enc.  The zero-bias tile zb (computed from the
        # last dec ACT's output) forces the scheduler to keep all dec ACTs
        # ahead of the enc ACTs - without it the static order interleaves
        # and ScalarE stalls on pe while dec work was ready.
        nc.scalar.activation(decb[:, :, 0, :], pd, Sin, scale=w0)
        nc.scalar.activation(decb[:, :, 1, :], pd, Sin, scale=w0, bias=nhpi)
        nc.scalar.activation(decb[:, :, 2, :], pd, Sin, scale=w1_)
        nc.scalar.activation(decb[:, :, 3, :], pd, Sin, scale=w1_, bias=nhpi)
        zb = const.tile([P, 1], F32, tag="zb")
        nc.scalar.activation(zb, decb[:, 0, 3, 0:1],
                             mybir.ActivationFunctionType.Copy, scale=0.0)
        nzhpi = const.tile([P, 1], F32, tag="nzhpi")
        nc.vector.tensor_scalar(nzhpi, zb, 1.0, -HALF_PI, op0=MULT, op1=ADD)
        nc.scalar.activation(encb[:, :, 0, :], pe, Sin, scale=w0, bias=zb)
        nc.scalar.activation(encb[:, :, 1, :], pe, Sin, scale=w0, bias=nzhpi)
        nc.scalar.activation(encb[:, :, 2, :], pe, Sin, scale=w1_, bias=zb)
        nc.scalar.activation(encb[:, :, 3, :], pe, Sin, scale=w1_, bias=nzhpi)
        # enc squares on ScalarE (idle after the base ACTs; Square is in
        # the trig table set so no table switch)
        Square = mybir.ActivationFunctionType.Square
        u1e = persist.tile([P, 2, ND], BF16, tag="u1e")
        nc.scalar.activation(u1e, encb[:, :, 2, :], Square)

        # NOTE: the ACT "cos" slots hold Sin(wx - pi/2) = -cos(wx); all the
        # sign flips are folded into the host-side vcn table columns:
        #   0: -c0*v   1: -c1*v   2: -2*c2*v   3: -c3*v   4: -2*c4*v
        def vc(kt, col):
            return vcn_sb[:, kt, col:col + 1]

        # ---- dec-side scaled factors ----
        # paS[m][sc]: [P, kt, EH] bf16, sc 0 pairs with enc-cos, 1 with enc-sin
        paS = persist.tile([P, 5, 2, 2, EH], BF16, tag="paS")

        for kt in range(2):
            nc.vector.tensor_scalar(paS[:, 0, kt, :, :], decb[:, kt, 0:2, :],
                                    vc(kt, 0), None, op0=MULT)
            nc.vector.tensor_scalar(paS[:, 1, kt, :, :], decb[:, kt, 2:4, :],
                                    vc(kt, 1), None, op0=MULT)

        # dec m2 = 2*b1: [u1|t12] = [s1|s1] * [s1|c1hat]
        #   u1 = sin^2, t12 = s*(-cos) = -sin(2t)/2
        dut = wrk.tile([P, 2, 2, EH], BF16, tag="dut")
        nc.vector.tensor_tensor(
            dut, decb[:, :, 2:3, :].broadcast_to([P, 2, 2, EH]),
            decb[:, :, 2:4, :], MULT)
        # dcos2 = cos(2t) = 1 - 2*u1 (true cosine; immediate TS is fast)
        dcos2 = wrk.tile([P, 2, EH], BF16, tag="dcos2")
        nc.vector.tensor_scalar(dcos2, dut[:, :, 0, :], -2.0, 1.0,
                                op0=MULT, op1=ADD)
        for kt in range(2):
            # sin2-scaled = t12*(-2v2) = v2*sin2 ; cos-side partner pairs
            # with enc t12e = -sin2b/2, so scale = -2v2 * cos2
            nc.vector.tensor_scalar(paS[:, 2, kt, 0, :], dut[:, kt, 1, :],
                                    vc(kt, 2), None, op0=MULT)
            nc.vector.tensor_scalar(paS[:, 2, kt, 1, :], dcos2[:, kt, :],
                                    vc(kt, 2), None, op0=MULT)

        # dec m3 = b0+b1: with chat = -cos slots,
        #   t1+t2 = s0*chat1 + chat0*s1 = -sin(S),  t3-t4 = chat0*chat1
        #   - s0*s1 = +cos(S)
        dt12 = wrk.tile([P, 2, 2, EH], BF16, tag="dt12")   # [t1|t2]
        dt43 = wrk.tile([P, 2, 2, EH], BF16, tag="dt43")   # [t4|t3]
        nc.vector.tensor_tensor(dt12[:, :, 0, :], decb[:, :, 0, :],
                                decb[:, :, 3, :], MULT)    # t1 = s0*chat1
        nc.vector.tensor_tensor(dt12[:, :, 1, :], decb[:, :, 1, :],
                                decb[:, :, 2, :], MULT)    # t2 = chat0*s1
        # [t4|t3] = [s0|chat0] * [s1|chat1]
        nc.vector.tensor_tensor(dt43, decb[:, :, 0:2, :],
                                decb[:, :, 2:4, :], MULT)
        dsc3 = wrk.tile([P, 2, 2, EH], BF16, tag="dsc3")   # [-sinS|+cosS]
        nc.vector.tensor_tensor(dsc3[:, :, 0, :], dt12[:, :, 0, :],
                                dt12[:, :, 1, :], ADD)
        nc.vector.tensor_tensor(dsc3[:, :, 1, :], dt43[:, :, 1, :],
                                dt43[:, :, 0, :], SUB)
        # dec m4 = 2*m3: [u3|t34] = [sin^2 S | -sin(2S)/2]
        dut3 = wrk.tile([P, 2, 2, EH], BF16, tag="dut3")
        nc.vector.tensor_tensor(
            dut3, dsc3[:, :, 0:1, :].broadcast_to([P, 2, 2, EH]),
            dsc3, MULT)
        dcos4 = wrk.tile([P, 2, EH], BF16, tag="dcos4")
        nc.vector.tensor_scalar(dcos4, dut3[:, :, 0, :], -2.0, 1.0,
                                op0=MULT, op1=ADD)
        for kt in range(2):
            nc.vector.tensor_scalar(paS[:, 3, kt, :, :], dsc3[:, kt, :, :],
                                    vc(kt, 3), None, op0=MULT)
            nc.vector.tensor_scalar(paS[:, 4, kt, 0, :], dut3[:, kt, 1, :],
                                    vc(kt, 4), None, op0=MULT)
            nc.vector.tensor_scalar(paS[:, 4, kt, 1, :], dcos4[:, kt, :],
                                    vc(kt, 4), None, op0=MULT)

        # ---- enc-side factors (true values; constant 2s folded into dec) ----
        # enc m2: t12e = s1*c1hat = -sin2/2 ; q2c = 1 - 2*u1e (u1e on ScalarE)
        t12e = persist.tile([P, 2, ND], BF16, tag="t12e")
        nc.vector.tensor_tensor(t12e, encb[:, :, 2, :],
                                encb[:, :, 3, :], MULT)
        q2c = persist.tile([P, 2, ND], BF16, tag="q2c")
        nc.vector.tensor_scalar(q2c, u1e, -2.0, 1.0, op0=MULT, op1=ADD)
        # enc m3: sum/diff chain
        et12 = wrk.tile([P, 2, 2, ND], BF16, tag="et12")
        et43 = wrk.tile([P, 2, 2, ND], BF16, tag="et43")
        nc.vector.tensor_tensor(et12[:, :, 0, :], encb[:, :, 0, :],
                                encb[:, :, 3, :], MULT)    # t1e = s0*c1
        nc.vector.tensor_tensor(et12[:, :, 1, :], encb[:, :, 1, :],
                                encb[:, :, 2, :], MULT)    # t2e = c0*s1
        nc.vector.tensor_tensor(et43, encb[:, :, 0:2, :],
                                encb[:, :, 2:4, :], MULT)  # [t4e|t3e]
        esc3 = persist.tile([P, 2, 2, ND], BF16, tag="esc3")  # [q3s|q3c]
        nc.vector.tensor_tensor(esc3[:, :, 0, :], et12[:, :, 0, :],
                                et12[:, :, 1, :], ADD)
        nc.vector.tensor_tensor(esc3[:, :, 1, :], et43[:, :, 1, :],
                                et43[:, :, 0, :], SUB)
        # enc m4: t34e = q3s*q3c = -sin(2S)/2 ; u3e = q3s^2 on ScalarE
        t34e = persist.tile([P, 2, ND], BF16, tag="t34e")
        nc.vector.tensor_tensor(t34e, esc3[:, :, 0, :],
                                esc3[:, :, 1, :], MULT)
        u3e = persist.tile([P, 2, ND], BF16, tag="u3e")
        nc.scalar.activation(u3e, esc3[:, :, 0, :], Square)
        q4c = persist.tile([P, 2, ND], BF16, tag="q4c")
        nc.vector.tensor_scalar(q4c, u3e, -2.0, 1.0, op0=MULT, op1=ADD)

        # ---- pair-product matmuls ----
        # per m: lhsT = paS[m,kt,0] x rhs = enc-cos ; paS[m,kt,1] x enc-sin
        # rhs (enc) tensors per m: (cos-factor, sin-factor)
        enc_rhs = [
            (encb[:, :, 1, :], encb[:, :, 0, :]),   # m0: c0hat, s0
            (encb[:, :, 3, :], encb[:, :, 2, :]),   # m1: c1hat, s1
            (q2c, t12e),                            # m2: cos2, -sin2/2
            (esc3[:, :, 1, :], esc3[:, :, 0, :]),   # m3: +cosS, -sinS
            (q4c, t34e),                            # m4: cos2S, -sin2S/2
        ]
        pbig = [psum.tile([P, ND], F32, tag=f"pbig{et}", name=f"pbig{et}")
                for et in range(2)]
        # et-major: half 0's chain (incl. its mask bias) completes before
        # half 1's final pairs so its softmax+store overlap them
        for et in range(2):
            for mi in range(5):
                rc, rs = enc_rhs[mi]
                for kt in range(2):
                    nc.tensor.matmul(
                        pbig[et],
                        lhsT=paS[:, mi, kt, 0, et * P:(et + 1) * P],
                        rhs=rc[:, kt, :],
                        start=(mi == 0 and kt == 0), stop=False,
                    )
                    nc.tensor.matmul(
                        pbig[et],
                        lhsT=paS[:, mi, kt, 1, et * P:(et + 1) * P],
                        rhs=rs[:, kt, :],
                        start=False, stop=False,
                    )
            # mask bias last (mask DMA off the critical path)
            nc.tensor.matmul(
                pbig[et],
                lhsT=id_sb,
                rhs=mk_sb[:, et, :],
                start=False, stop=True,
            )

        # preload the exp table set behind the final pair matmuls
        warm2 = const.tile([P, 1], F32, tag="warm2")
        nc.scalar.activation(warm2, encb[:, 0, 3, 0:1], Exp)

        # ---- masked softmax over d (free axis) ----
        for et in range(2):
            expv = wrk.tile([P, ND], F32, tag="expv", name=f"expv{et}")
            zsum = wrk.tile([P, 1], F32, tag="zsum", name=f"zsum{et}")
            nc.scalar.activation(expv, pbig[et], Exp, accum_out=zsum)
            rz = wrk.tile([P, 1], F32, tag="rz", name=f"rz{et}")
            nc.vector.reciprocal(rz, zsum)
            outv = wrk.tile([P, ND], BF16, tag="outv", name=f"outv{et}")
            nc.vector.tensor_scalar(outv, expv, rz, None, op0=MULT)
            nc.sync.dma_start(out=out.ap()[:, et, :], in_=outv)

    if finalize:
        nc.finalize()
    return nc


_PROGRAM = None


def _get_program():
    global _PROGRAM
    if _PROGRAM is None:
        _PROGRAM = _build_program()
    return _PROGRAM


def build_in_maps(x_decoder, x_encoder, mask, w1, w2, v):
    import ml_dtypes
    bf = ml_dtypes.bfloat16
    x_decoder = np.asarray(x_decoder, dtype=np.float32)
    x_encoder = np.asarray(x_encoder, dtype=np.float32)
    mask = np.asarray(mask)
    w1 = np.asarray(w1, dtype=np.float32)
    w2 = np.asarray(w2, dtype=np.float32)
    v = np.asarray(v, dtype=np.float32)

    def pm(mat, cols):
        """[C, cols] -> partition-major [P, 2, cols] (c = ct*128 + p)."""
        return np.ascontiguousarray(
            mat.reshape(2, P, cols).transpose(1, 0, 2)).astype(bf)

    w1p = pm(np.ascontiguousarray(w1.T), C)    # [P, ct, k]
    w2p = pm(np.ascontiguousarray(w2.T), C)
    xep_all = {}
    for b in range(B):
        xep_all[b] = pm(np.ascontiguousarray(x_encoder[b].T), ND)

    # vcn[p, kt, col], k = kt*128 + p; signs absorb the -cos ACT slots
    cf = [np.float32(c) for c in COEFS]
    vcn = np.empty((P, 2, 5), dtype=np.float32)
    for kt in range(2):
        vk = v[kt * P:(kt + 1) * P]
        vcn[:, kt, 0] = -cf[0] * vk
        vcn[:, kt, 1] = -cf[1] * vk
        vcn[:, kt, 2] = -2 * cf[2] * vk
        vcn[:, kt, 3] = -cf[3] * vk
        vcn[:, kt, 4] = -2 * cf[4] * vk

    identity = np.eye(P, dtype=np.float32).astype(bf)
    in_maps = []
    for core in range(NCORES):
        b, h = divmod(core, 2)
        sl = slice(h * EH, (h + 1) * EH)
        xdp = pm(np.ascontiguousarray(x_decoder[b, sl, :].T), EH)  # [P,ct,e]
        blob1 = np.concatenate(
            [xdp.reshape(P, 512), w2p.reshape(P, 512)], axis=1)
        blob2 = np.concatenate(
            [w1p.reshape(P, 512), xep_all[b].reshape(P, 1024)], axis=1)
        mskp = (mask[b, sl, :].astype(np.float32) * np.float32(MASK_SCALE)
                ).reshape(2, P, ND).transpose(1, 0, 2)  # e = et*128 + p
        in_maps.append({
            "blob1": np.ascontiguousarray(blob1),
            "blob2": np.ascontiguousarray(blob2),
            "vcn": vcn,
            "msk": np.ascontiguousarray(mskp).astype(bf),
            "ident": identity,
        })
    return in_maps


def kernel(x_decoder, x_encoder, mask, w1, w2, v):
    in_maps = build_in_maps(x_decoder, x_encoder, mask, w1, w2, v)
    nc = _get_program()
    res = run_bass_kernel_spmd(nc, in_maps, core_ids=list(range(NCORES)))

    out = np.empty((B, NE, ND), dtype=np.float32)
    for core in range(NCORES):
        b, h = divmod(core, 2)
        o = res.results[core]["out"].astype(np.float32)  # [P, 2, ND]
        out[b, h * EH:(h + 1) * EH, :] = \
            o.transpose(1, 0, 2).reshape(EH, ND)
    return out
